# revision 9
# baseline (speedup 1.0000x reference)
"""Trainium2 Bass kernel for nn_Attention (dense transformer attention block).

Computation (per reference):
  q = x @ wq.T; k = x @ wk.T; v = x @ wv.T       (GQA: 16 q heads, 4 kv heads)
  rope(q, k) with cos/sin from freqs (interleaved complex pairs)
  non-causal SDPA with softmax over keys, scale 1/sqrt(128)
  out = (probs @ v reshaped) @ wo.T

Sharding (8 cores): tensor-parallel over the 4 kv-head groups (TP=4; each
core gets 4 q heads + 1 kv head, wq/wk/wv column-sharded, wo row-sharded)
x data-parallel over batch (DP=2; 2 batches per core). Each core computes a
partial output [2, S, DIM]; the host sums the 4 TP partials per batch pair.

Device layout notes:
 - x is passed transposed per batch: xt[b] = x[b].T  [DIM, S] so that
   projections contract over DIM on the partition axis.
 - Q^T/K^T are computed in [head_dim, S] layout; per head the 128
   head-dim rows are permuted to [evens(64) | odds(64)] (done by permuting
   wq/wk rows on the host) so RoPE pairs are partition-contiguous halves.
   RoPE runs as 2 full-width products + 2 half-width combine ops against
   partition-duplicated cos/sin tables.
 - scores are computed transposed: S^T[k, q] = K^T.T @ Q^T per 128k x 512q
   tile; exp via ACT (scale folded in). Phase B is software-pipelined: the
   AV / denominator matmuls for score-group kg are emitted one group late
   so the in-order PE queue never stalls on the ACT exp.
 - softmax denominators: 4 col-tiled ones-matmuls (M=32 broadcast rows,
   emitted adjacently so they pack into the PE array concurrently)
   accumulate per-column partial sums; a selection-matrix matmul sums the
   4 partials and broadcasts to 128 partitions; reciprocal on DVE; the AV
   output is scaled by it.
 - AV output is produced in [head_dim, S] layout which directly feeds the
   wo matmul as lhsT.
 - matmuls run in fp32r (full PE rate, ~1e-4 rel err); the attention
   probabilities/AV/wo-side uses bf16 operands (errors ~0.3%).
"""

import numpy as np
from contextlib import ExitStack

import ml_dtypes

import concourse.bacc as bacc
import concourse.tile as tile
from concourse import mybir
from concourse.bass_utils import run_bass_kernel_spmd
from concourse.masks import make_identity

F32 = mybir.dt.float32
F32R = mybir.dt.float32r
BF16 = mybir.dt.bfloat16

N_HEADS = 16
N_KV_HEADS = 4
DIM = 2048
HD = 128
B = 4
S_FULL = 2048
TP = 4            # tensor-parallel over kv-head groups
DP = 2            # data-parallel over batch
BPC = B // DP     # batches per core
HQ = N_HEADS // TP  # q heads per core
DK = DIM // 128   # contraction tiles over model dim
SCALE = float(1.0 / np.sqrt(HD))

_NC_CACHE = {}


def build_nc(s):
    sc_n = s // 512   # 512-wide s/q chunks
    kt_n = s // 128   # 128-wide key tiles
    st_n = s // 128   # 128-wide s tiles
    kg_n = kt_n // 2  # score groups (2 key tiles each)

    nc = bacc.Bacc("TRN2", target_bir_lowering=False, debug=False)
    xt = nc.dram_tensor("xt", [BPC, DIM, s], F32, kind="ExternalInput")
    cost = nc.dram_tensor("cost", [BPC, 64, s], F32, kind="ExternalInput")
    sint = nc.dram_tensor("sint", [BPC, 64, s], F32, kind="ExternalInput")
    wqt = nc.dram_tensor("wqt", [DIM, HQ * HD], F32, kind="ExternalInput")
    wkt = nc.dram_tensor("wkt", [DIM, HD], F32, kind="ExternalInput")
    wvt = nc.dram_tensor("wvt", [DIM, HD], F32, kind="ExternalInput")
    wot = nc.dram_tensor("wot", [HQ * HD, DIM], BF16, kind="ExternalInput")
    outp = nc.dram_tensor("outp", [BPC, s, DIM], F32, kind="ExternalOutput")

    wqt_v = wqt.rearrange("(dk p) c -> dk p c", p=128)
    wkt_v = wkt.rearrange("(dk p) c -> dk p c", p=128)
    wvt_v = wvt.rearrange("(dk p) c -> dk p c", p=128)
    wot_v = wot.rearrange("(h p) c -> h p c", p=128)

    with ExitStack() as ctx:
        ctx.enter_context(
            nc.allow_low_precision(reason="fp32r/bf16 matmul pipeline by design")
        )
        tc = ctx.enter_context(tile.TileContext(nc))

        singles = ctx.enter_context(tc.tile_pool(name="singles", bufs=1))
        qt_pool = ctx.enter_context(tc.tile_pool(name="qt", bufs=1))
        kt_pool = ctx.enter_context(tc.tile_pool(name="ktp", bufs=1))
        v_pool = ctx.enter_context(tc.tile_pool(name="vp", bufs=1))
        e_pool = ctx.enter_context(tc.tile_pool(name="ep", bufs=1))
        ot_pool = ctx.enter_context(tc.tile_pool(name="otp", bufs=1))
        cs_pool = ctx.enter_context(tc.tile_pool(name="csp", bufs=2))
        xt_pool = ctx.enter_context(tc.tile_pool(name="xtp", bufs=3))
        tmp_pool = ctx.enter_context(tc.tile_pool(name="tmp", bufs=2))
        vt_pool = ctx.enter_context(tc.tile_pool(name="vtp", bufs=2))
        csum_pool = ctx.enter_context(tc.tile_pool(name="csum", bufs=2))
        rcp_pool = ctx.enter_context(tc.tile_pool(name="rcp", bufs=2))
        av_pool = ctx.enter_context(tc.tile_pool(name="avp", bufs=2))
        orow_pool = ctx.enter_context(tc.tile_pool(name="orow", bufs=2))

        psum = ctx.enter_context(tc.tile_pool(name="psum", bufs=4, space="PSUM"))

        def ps_tile(name):
            return psum.tile([128, 2, 512], F32, tag="pair", name=name)

        # ---- weights / constants (resident) ----
        wk_sb = singles.tile([128, DK, HD], F32R)
        wv_sb = singles.tile([128, DK, HD], F32R)
        wq_sb = singles.tile([128, DK, HQ * HD], F32R)
        for dk in range(DK):
            nc.sync.dma_start(out=wk_sb[:, dk, :], in_=wkt_v[dk].bitcast(F32R))
            nc.sync.dma_start(out=wv_sb[:, dk, :], in_=wvt_v[dk].bitcast(F32R))
            nc.sync.dma_start(out=wq_sb[:, dk, :], in_=wqt_v[dk].bitcast(F32R))
        wo_sb = singles.tile([128, HQ, DIM], BF16)
        wo_loaded = [False]

        def load_wo():
            if not wo_loaded[0]:
                for h in range(HQ):
                    nc.sync.dma_start(out=wo_sb[:, h, :], in_=wot_v[h])
                wo_loaded[0] = True

        ones32_bf = singles.tile([128, 32], BF16)
        nc.vector.memset(ones32_bf, 1.0)
        sel4_f = singles.tile([128, 128], F32)
        nc.vector.memset(sel4_f, 0.0)
        for j in range(4):
            nc.vector.memset(sel4_f[32 * j:32 * j + 1, :], 1.0)
        sel4 = singles.tile([128, 128], F32R)
        nc.vector.tensor_copy(sel4, sel4_f)
        ident = singles.tile([128, 128], F32)
        make_identity(nc, ident)

        copy_flip = [0]

        def copy_any(dst, src):
            # alternate psum->sbuf copies between ScalarE and VectorE
            if copy_flip[0] % 2 == 0:
                nc.scalar.copy(dst, src)
            else:
                nc.vector.tensor_copy(dst, src)
            copy_flip[0] += 1

        for b in range(BPC):
            # ---- phase A: projections + rope ----
            # cos2/sin2: [128, s] with the 64 rope rows duplicated into both
            # partition halves (two DMAs from the same DRAM source)
            cos2 = cs_pool.tile([128, s], F32, tag="cs")
            sin2 = cs_pool.tile([128, s], F32, tag="cs")

            qt = qt_pool.tile([128, HQ, s], F32R)
            kt = kt_pool.tile([128, s], F32R)
            vsb = v_pool.tile([128, st_n, HD], BF16)

            for sc in range(sc_n):
                ss = slice(sc * 512, (sc + 1) * 512)
                qps = [ps_tile(f"qps{i}") for i in range(2)]
                kvps = ps_tile("kvps")
                for t in range(4):
                    xtile = xt_pool.tile([128, 4, 512], F32R)
                    for dkl in range(4):
                        dk = 4 * t + dkl
                        nc.sync.dma_start(
                            out=xtile[:, dkl, :],
                            in_=xt[b, dk * 128:(dk + 1) * 128, ss].bitcast(F32R),
                        )
                    if sc == 0 and t == 0:
                        for half in range(2):
                            nc.sync.dma_start(
                                out=cos2[64 * half:64 * (half + 1), :], in_=cost[b]
                            )
                            nc.sync.dma_start(
                                out=sin2[64 * half:64 * (half + 1), :], in_=sint[b]
                            )
                    for dkl in range(4):
                        dk = 4 * t + dkl
                        st_ = dk == 0
                        sp_ = dk == DK - 1
                        rhs = xtile[:, dkl, :]
                        for m in range(HQ):
                            nc.tensor.matmul(
                                qps[m // 2][:, m % 2, :],
                                wq_sb[:, dk, m * HD:(m + 1) * HD],
                                rhs,
                                start=st_,
                                stop=sp_,
                            )
                        nc.tensor.matmul(
                            kvps[:, 0, :], wk_sb[:, dk, :], rhs, start=st_, stop=sp_
                        )
                        nc.tensor.matmul(
                            kvps[:, 1, :], wv_sb[:, dk, :], rhs, start=st_, stop=sp_
                        )

                # rope: r' = qr*cos - qi*sin ; i' = qr*sin + qi*cos
                # P1 = [qr;qi] * [cos;cos], P2 = [qr;qi] * [sin;sin]
                # r' = P1[top] - P2[bot] ; i' = P2[top] + P1[bot]
                def rope(src_ps, dst_r, dst_i):
                    # p1 = src*cos (SBUF); then src *= sin in place (PSUM) —
                    # two SBUF TT inputs must share a base partition, but a
                    # PSUM input may sit at any base, so the sin product
                    # stays in the source psum.
                    p1 = tmp_pool.tile([128, 512], F32, tag="tmp")
                    nc.vector.tensor_mul(p1, src_ps, cos2[:, ss])
                    nc.vector.tensor_mul(src_ps, src_ps, sin2[:, ss])
                    nc.vector.tensor_sub(dst_r, p1[0:64, :], src_ps[64:128, :])
                    nc.vector.tensor_add(dst_i, src_ps[0:64, :], p1[64:128, :])

                # K first (its rope frees kvps for the next chunk soonest),
                # V-chain on ACT/PE only, then the Q ropes on DVE.
                rope(kvps[:, 0, :], kt[0:64, ss], kt[64:128, ss])

                # V: copy psum -> sbuf, transpose 128x128 blocks back into the
                # same psum slice, copy out as [s, d] bf16
                vt_sb = vt_pool.tile([128, 512], F32, tag="vt")
                nc.scalar.copy(vt_sb, kvps[:, 1, :])
                for i in range(4):
                    vtr = kvps[:, 1, i * 128:(i + 1) * 128]
                    nc.tensor.transpose(vtr, vt_sb[:, i * 128:(i + 1) * 128], ident)
                    nc.scalar.copy(vsb[:, sc * 4 + i, :], vtr)

                for m in range(HQ):
                    rope(
                        qps[m // 2][:, m % 2, :],
                        qt[0:64, m, ss],
                        qt[64:128, m, ss],
                    )

            # ---- phase B: attention per head (software-pipelined) ----
            outT = ot_pool.tile([128, HQ, s], BF16)
            finalize_prev = [None]

            for h in range(HQ):
                for qc in range(sc_n):
                    qs = slice(qc * 512, (qc + 1) * 512)
                    e_t = e_pool.tile([128, kt_n, 512], BF16)
                    avsm = [None]  # allocated at kg==1 (reuses finalized slot)

                    def emit_av(kg, avsm=avsm, e_t=e_t):
                        for j in range(2):
                            ktile = 2 * kg + j
                            nc.tensor.matmul(
                                avsm[0][:, 0, :],
                                vsb[:, ktile, :],
                                e_t[:, ktile, :],
                                start=(ktile == 0),
                                stop=(ktile == kt_n - 1),
                            )

                    def emit_colsums(i4, avsm=avsm, e_t=e_t):
                        # 4 adjacent col-tiled ones-matmuls (pack concurrently)
                        for cj in range(4):
                            ktile = 4 * i4 + cj
                            nc.tensor.matmul(
                                avsm[0][32 * cj:32 * (cj + 1), 1, :],
                                ones32_bf,
                                e_t[:, ktile, :],
                                start=(i4 == 0),
                                stop=(i4 == kt_n // 4 - 1),
                                tile_position=(0, 32 * cj),
                            )

                    for kg in range(kg_n):
                        sc_ps = ps_tile("scps")
                        for j in range(2):
                            ktile = 2 * kg + j
                            nc.tensor.matmul(
                                sc_ps[:, j, :],
                                kt[:, ktile * 128:(ktile + 1) * 128],
                                qt[:, h, qs],
                                start=True,
                                stop=True,
                            )
                        nc.scalar.activation(
                            out=e_t[:, 2 * kg:2 * kg + 2, :],
                            in_=sc_ps,
                            func=mybir.ActivationFunctionType.Exp,
                            scale=SCALE,
                        )
                        if kg == 0 and finalize_prev[0] is not None:
                            finalize_prev[0]()
                            finalize_prev[0] = None
                        if kg == 1:
                            avsm[0] = ps_tile("avsm")
                        if kg >= 1:
                            emit_av(kg - 1)
                            if kg % 2 == 0:
                                emit_colsums(kg // 2 - 1)
                    emit_av(kg_n - 1)
                    emit_colsums(kg_n // 2 - 1)

                    def finalize(avsm=avsm, h=h, qs=qs):
                        av_t = avsm[0]
                        csum = csum_pool.tile([128, 512], F32R, tag="csum")
                        nc.vector.tensor_copy(csum, av_t[:, 1, :])
                        av_sb = av_pool.tile([128, 512], BF16, tag="avsb")
                        nc.scalar.copy(av_sb, av_t[:, 0, :])
                        bc_ps = ps_tile("bcps")
                        nc.tensor.matmul(
                            bc_ps[:, 0, :], sel4, csum, start=True, stop=True
                        )
                        rcp = rcp_pool.tile([128, 512], F32, tag="rcp")
                        nc.vector.reciprocal_approx_fast(
                            out=rcp, in_=bc_ps[:, 0, :]
                        )
                        nc.vector.tensor_mul(outT[:, h, qs], av_sb, rcp)

                    finalize_prev[0] = finalize
            finalize_prev[0]()

            # ---- phase C: output projection ----
            load_wo()
            for scb in range(st_n):
                sb_ = slice(scb * 128, (scb + 1) * 128)
                for dc in range(0, DIM // 512, 2):
                    ops_ = ps_tile("ops")
                    for jj in range(2):
                        for h2 in range(HQ):
                            nc.tensor.matmul(
                                ops_[:, jj, :],
                                outT[:, h2, sb_],
                                wo_sb[:, h2, (dc + jj) * 512:(dc + jj + 1) * 512],
                                start=(h2 == 0),
                                stop=(h2 == HQ - 1),
                            )
                    orow = orow_pool.tile([128, 2, 512], F32, tag="orow")
                    copy_any(orow, ops_)
                    nc.sync.dma_start(
                        out=outp[b, sb_, dc * 512:(dc + 2) * 512],
                        in_=orow.rearrange("p a b -> p (a b)"),
                    )

    nc.compile()
    return nc


_PERM = np.concatenate([np.arange(0, HD, 2), np.arange(1, HD, 2)])


def _prep_inputs(x, freqs, wq, wk, wv, wo, s):
    """Build the 8 per-core input maps."""
    in_maps = []
    xt_dp = []
    cos_dp = []
    sin_dp = []
    for dp in range(DP):
        bs = slice(dp * BPC, (dp + 1) * BPC)
        xt_dp.append(np.ascontiguousarray(x[bs].transpose(0, 2, 1)))
        cos_dp.append(np.ascontiguousarray(np.cos(freqs[bs]).transpose(0, 2, 1)))
        sin_dp.append(np.ascontiguousarray(np.sin(freqs[bs]).transpose(0, 2, 1)))
    for core in range(8):
        g = core % TP
        dp = core // TP
        wq_g = wq[g * HQ * HD:(g + 1) * HQ * HD]  # [512, DIM]
        wq_p = wq_g.reshape(HQ, HD, DIM)[:, _PERM, :].reshape(HQ * HD, DIM)
        wk_g = wk[g * HD:(g + 1) * HD][_PERM]      # [128, DIM]
        wv_g = wv[g * HD:(g + 1) * HD]             # [128, DIM]
        wo_g = wo[:, g * HQ * HD:(g + 1) * HQ * HD]  # [DIM, 512]
        in_maps.append(
            {
                "xt": xt_dp[dp],
                "cost": cos_dp[dp],
                "sint": sin_dp[dp],
                "wqt": np.ascontiguousarray(wq_p.T),
                "wkt": np.ascontiguousarray(wk_g.T),
                "wvt": np.ascontiguousarray(wv_g.T),
                "wot": np.ascontiguousarray(wo_g.T).astype(ml_dtypes.bfloat16),
            }
        )
    return in_maps


_LAST = {}


def _run(x, freqs, wq, wk, wv, wo, s):
    x = np.asarray(x, dtype=np.float32)
    freqs = np.asarray(freqs, dtype=np.float32)
    wq = np.asarray(wq, dtype=np.float32)
    wk = np.asarray(wk, dtype=np.float32)
    wv = np.asarray(wv, dtype=np.float32)
    wo = np.asarray(wo, dtype=np.float32)

    if s not in _NC_CACHE:
        _NC_CACHE[s] = build_nc(s)
    nc = _NC_CACHE[s]
    in_maps = _prep_inputs(x, freqs, wq, wk, wv, wo, s)
    res = run_bass_kernel_spmd(nc, in_maps, core_ids=list(range(8)))
    _LAST["nc"] = nc
    _LAST["in_maps"] = in_maps

    out = np.empty((B, s, DIM), dtype=np.float32)
    for dp in range(DP):
        acc = res.results[dp * TP]["outp"].copy()
        for g in range(1, TP):
            acc += res.results[dp * TP + g]["outp"]
        out[dp * BPC:(dp + 1) * BPC] = acc
    return out


def kernel(x, freqs, wq, wk, wv, wo):
    return _run(x, freqs, wq, wk, wv, wo, S_FULL)


# revision 10
# speedup vs baseline: 1.0238x; 1.0238x over previous
"""Trainium2 Bass kernel for nn_Attention (dense transformer attention block).

Computation (per reference):
  q = x @ wq.T; k = x @ wk.T; v = x @ wv.T       (GQA: 16 q heads, 4 kv heads)
  rope(q, k) with cos/sin from freqs (interleaved complex pairs)
  non-causal SDPA with softmax over keys, scale 1/sqrt(128)
  out = (probs @ v reshaped) @ wo.T

Sharding (8 cores): tensor-parallel over the 4 kv-head groups (TP=4; each
core gets 4 q heads + 1 kv head, wq/wk/wv column-sharded, wo row-sharded)
x data-parallel over batch (DP=2; 2 batches per core). Each core computes a
partial output [2, S, DIM]; the host sums the 4 TP partials per batch pair.

Device layout notes:
 - x is passed transposed per batch: xt[b] = x[b].T  [DIM, S] so that
   projections contract over DIM on the partition axis.
 - Q^T/K^T are computed in [head_dim, S] layout; per head the 128
   head-dim rows are permuted to [evens(64) | odds(64)] (done by permuting
   wq/wk rows on the host) so RoPE pairs are partition-contiguous halves.
   RoPE runs as 2 full-width products + 2 half-width combine ops against
   partition-duplicated cos/sin tables.
 - scores are computed transposed: S^T[k, q] = K^T.T @ Q^T per 128k x 512q
   tile; exp via ACT (scale folded in). Phase B is software-pipelined: the
   AV / denominator matmuls for score-group kg are emitted one group late
   so the in-order PE queue never stalls on the ACT exp.
 - softmax denominators: 4 col-tiled ones-matmuls (M=32 broadcast rows,
   emitted adjacently so they pack into the PE array concurrently)
   accumulate per-column partial sums; a selection-matrix matmul sums the
   4 partials and broadcasts to 128 partitions; reciprocal on DVE; the AV
   output is scaled by it.
 - AV output is produced in [head_dim, S] layout which directly feeds the
   wo matmul as lhsT.
 - matmuls run in fp32r (full PE rate, ~1e-4 rel err); the attention
   probabilities/AV/wo-side uses bf16 operands (errors ~0.3%).
"""

import numpy as np
from contextlib import ExitStack

import ml_dtypes

import concourse.bacc as bacc
import concourse.tile as tile
from concourse import mybir
from concourse.bass_utils import run_bass_kernel_spmd
from concourse.masks import make_identity

F32 = mybir.dt.float32
F32R = mybir.dt.float32r
BF16 = mybir.dt.bfloat16

N_HEADS = 16
N_KV_HEADS = 4
DIM = 2048
HD = 128
B = 4
S_FULL = 2048
TP = 4            # tensor-parallel over kv-head groups
DP = 2            # data-parallel over batch
BPC = B // DP     # batches per core
HQ = N_HEADS // TP  # q heads per core
DK = DIM // 128   # contraction tiles over model dim
SCALE = float(1.0 / np.sqrt(HD))

_NC_CACHE = {}


def build_nc(s):
    sc_n = s // 512   # 512-wide s/q chunks
    kt_n = s // 128   # 128-wide key tiles
    st_n = s // 128   # 128-wide s tiles
    kg_n = kt_n // 2  # score groups (2 key tiles each)

    nc = bacc.Bacc("TRN2", target_bir_lowering=False, debug=False)
    xt = nc.dram_tensor("xt", [BPC, DIM, s], F32, kind="ExternalInput")
    cost = nc.dram_tensor("cost", [BPC, 64, s], F32, kind="ExternalInput")
    sint = nc.dram_tensor("sint", [BPC, 64, s], F32, kind="ExternalInput")
    wqt = nc.dram_tensor("wqt", [DIM, HQ * HD], F32, kind="ExternalInput")
    wkt = nc.dram_tensor("wkt", [DIM, HD], F32, kind="ExternalInput")
    wvt = nc.dram_tensor("wvt", [DIM, HD], F32, kind="ExternalInput")
    wot = nc.dram_tensor("wot", [HQ * HD, DIM], BF16, kind="ExternalInput")
    outp = nc.dram_tensor("outp", [BPC, s, DIM], F32, kind="ExternalOutput")

    wqt_v = wqt.rearrange("(dk p) c -> dk p c", p=128)
    wkt_v = wkt.rearrange("(dk p) c -> dk p c", p=128)
    wvt_v = wvt.rearrange("(dk p) c -> dk p c", p=128)
    wot_v = wot.rearrange("(h p) c -> h p c", p=128)

    with ExitStack() as ctx:
        ctx.enter_context(
            nc.allow_low_precision(reason="fp32r/bf16 matmul pipeline by design")
        )
        tc = ctx.enter_context(tile.TileContext(nc))

        singles = ctx.enter_context(tc.tile_pool(name="singles", bufs=1))
        qt_pool = ctx.enter_context(tc.tile_pool(name="qt", bufs=1))
        kt_pool = ctx.enter_context(tc.tile_pool(name="ktp", bufs=1))
        v_pool = ctx.enter_context(tc.tile_pool(name="vp", bufs=1))
        e_pool = ctx.enter_context(tc.tile_pool(name="ep", bufs=1))
        ot_pool = ctx.enter_context(tc.tile_pool(name="otp", bufs=1))
        cs_pool = ctx.enter_context(tc.tile_pool(name="csp", bufs=2))
        xt_pool = ctx.enter_context(tc.tile_pool(name="xtp", bufs=3))
        tmp_pool = ctx.enter_context(tc.tile_pool(name="tmp", bufs=2))
        vt_pool = ctx.enter_context(tc.tile_pool(name="vtp", bufs=2))
        csum_pool = ctx.enter_context(tc.tile_pool(name="csum", bufs=2))
        rcp_pool = ctx.enter_context(tc.tile_pool(name="rcp", bufs=2))
        av_pool = ctx.enter_context(tc.tile_pool(name="avp", bufs=2))
        orow_pool = ctx.enter_context(tc.tile_pool(name="orow", bufs=2))

        psum = ctx.enter_context(tc.tile_pool(name="psum", bufs=4, space="PSUM"))

        def ps_tile(name):
            return psum.tile([128, 2, 512], F32, tag="pair", name=name)

        # ---- weights / constants (resident) ----
        wk_sb = singles.tile([128, DK, HD], F32R)
        wv_sb = singles.tile([128, DK, HD], F32R)
        wq_sb = singles.tile([128, DK, HQ * HD], F32R)
        for dk in range(DK):
            nc.sync.dma_start(out=wk_sb[:, dk, :], in_=wkt_v[dk].bitcast(F32R))
            nc.sync.dma_start(out=wv_sb[:, dk, :], in_=wvt_v[dk].bitcast(F32R))
            nc.sync.dma_start(out=wq_sb[:, dk, :], in_=wqt_v[dk].bitcast(F32R))
        wo_sb = singles.tile([128, HQ, DIM], BF16)
        wo_loaded = [False]

        def load_wo():
            if not wo_loaded[0]:
                for h in range(HQ):
                    nc.sync.dma_start(out=wo_sb[:, h, :], in_=wot_v[h])
                wo_loaded[0] = True

        ones32_bf = singles.tile([128, 32], BF16)
        nc.vector.memset(ones32_bf, 1.0)
        sel4_f = singles.tile([128, 128], F32)
        nc.vector.memset(sel4_f, 0.0)
        for j in range(4):
            nc.vector.memset(sel4_f[32 * j:32 * j + 1, :], 1.0)
        sel4 = singles.tile([128, 128], F32R)
        nc.vector.tensor_copy(sel4, sel4_f)
        ident = singles.tile([128, 128], F32)
        make_identity(nc, ident)

        copy_flip = [0]

        def copy_any(dst, src):
            # alternate psum->sbuf copies between ScalarE and VectorE
            if copy_flip[0] % 2 == 0:
                nc.scalar.copy(dst, src)
            else:
                nc.vector.tensor_copy(dst, src)
            copy_flip[0] += 1

        for b in range(BPC):
            # ---- phase A: projections + rope ----
            # cos2/sin2: [128, s] with the 64 rope rows duplicated into both
            # partition halves (two DMAs from the same DRAM source)
            cos2 = cs_pool.tile([128, s], F32, tag="cs")
            sin2 = cs_pool.tile([128, s], F32, tag="cs")

            qt = qt_pool.tile([128, HQ, s], F32R)
            kt = kt_pool.tile([128, s], F32R)
            vsb = v_pool.tile([128, st_n, HD], BF16)

            for sc in range(sc_n):
                ss = slice(sc * 512, (sc + 1) * 512)
                qps = [ps_tile(f"qps{i}") for i in range(2)]
                kvps = ps_tile("kvps")
                for t in range(4):
                    xtile = xt_pool.tile([128, 4, 512], F32R)
                    for dkl in range(4):
                        dk = 4 * t + dkl
                        nc.sync.dma_start(
                            out=xtile[:, dkl, :],
                            in_=xt[b, dk * 128:(dk + 1) * 128, ss].bitcast(F32R),
                        )
                    if sc == 0 and t == 0:
                        for half in range(2):
                            nc.sync.dma_start(
                                out=cos2[64 * half:64 * (half + 1), :], in_=cost[b]
                            )
                            nc.sync.dma_start(
                                out=sin2[64 * half:64 * (half + 1), :], in_=sint[b]
                            )
                    for dkl in range(4):
                        dk = 4 * t + dkl
                        st_ = dk == 0
                        sp_ = dk == DK - 1
                        rhs = xtile[:, dkl, :]
                        for m in range(HQ):
                            nc.tensor.matmul(
                                qps[m // 2][:, m % 2, :],
                                wq_sb[:, dk, m * HD:(m + 1) * HD],
                                rhs,
                                start=st_,
                                stop=sp_,
                            )
                        nc.tensor.matmul(
                            kvps[:, 0, :], wk_sb[:, dk, :], rhs, start=st_, stop=sp_
                        )
                        nc.tensor.matmul(
                            kvps[:, 1, :], wv_sb[:, dk, :], rhs, start=st_, stop=sp_
                        )

                # rope: r' = qr*cos - qi*sin ; i' = qr*sin + qi*cos
                # P1 = [qr;qi] * [cos;cos], P2 = [qr;qi] * [sin;sin]
                # r' = P1[top] - P2[bot] ; i' = P2[top] + P1[bot]
                def rope(src_ps, dst_r, dst_i):
                    # p1 = src*cos (SBUF); then src *= sin in place (PSUM) —
                    # two SBUF TT inputs must share a base partition, but a
                    # PSUM input may sit at any base, so the sin product
                    # stays in the source psum.
                    p1 = tmp_pool.tile([128, 512], F32, tag="tmp")
                    nc.vector.tensor_mul(p1, src_ps, cos2[:, ss])
                    nc.vector.tensor_mul(src_ps, src_ps, sin2[:, ss])
                    nc.vector.tensor_sub(dst_r, p1[0:64, :], src_ps[64:128, :])
                    nc.vector.tensor_add(dst_i, src_ps[0:64, :], p1[64:128, :])

                for m in range(HQ):
                    rope(
                        qps[m // 2][:, m % 2, :],
                        qt[0:64, m, ss],
                        qt[64:128, m, ss],
                    )
                rope(kvps[:, 0, :], kt[0:64, ss], kt[64:128, ss])

                # V: copy psum -> sbuf, transpose 128x128 blocks back into the
                # same psum slice, copy out as [s, d] bf16 (ACT-only copies —
                # DVE is busy with the ropes)
                vt_sb = vt_pool.tile([128, 512], F32, tag="vt")
                nc.scalar.copy(vt_sb, kvps[:, 1, :])
                for i in range(4):
                    vtr = kvps[:, 1, i * 128:(i + 1) * 128]
                    nc.tensor.transpose(vtr, vt_sb[:, i * 128:(i + 1) * 128], ident)
                    nc.scalar.copy(vsb[:, sc * 4 + i, :], vtr)

            # ---- phase B: attention per head (software-pipelined) ----
            outT = ot_pool.tile([128, HQ, s], BF16)
            finalize_prev = [None]

            for h in range(HQ):
                for qc in range(sc_n):
                    qs = slice(qc * 512, (qc + 1) * 512)
                    e_t = e_pool.tile([128, kt_n, 512], BF16)
                    avsm = [None]  # allocated at kg==1 (reuses finalized slot)

                    def emit_av(kg, avsm=avsm, e_t=e_t):
                        for j in range(2):
                            ktile = 2 * kg + j
                            nc.tensor.matmul(
                                avsm[0][:, 0, :],
                                vsb[:, ktile, :],
                                e_t[:, ktile, :],
                                start=(ktile == 0),
                                stop=(ktile == kt_n - 1),
                            )

                    def emit_colsums(i4, avsm=avsm, e_t=e_t):
                        # 4 adjacent col-tiled ones-matmuls (pack concurrently)
                        for cj in range(4):
                            ktile = 4 * i4 + cj
                            nc.tensor.matmul(
                                avsm[0][32 * cj:32 * (cj + 1), 1, :],
                                ones32_bf,
                                e_t[:, ktile, :],
                                start=(i4 == 0),
                                stop=(i4 == kt_n // 4 - 1),
                                tile_position=(0, 32 * cj),
                            )

                    for kg in range(kg_n):
                        sc_ps = ps_tile("scps")
                        for j in range(2):
                            ktile = 2 * kg + j
                            nc.tensor.matmul(
                                sc_ps[:, j, :],
                                kt[:, ktile * 128:(ktile + 1) * 128],
                                qt[:, h, qs],
                                start=True,
                                stop=True,
                            )
                        nc.scalar.activation(
                            out=e_t[:, 2 * kg:2 * kg + 2, :],
                            in_=sc_ps,
                            func=mybir.ActivationFunctionType.Exp,
                            scale=SCALE,
                        )
                        if kg == 0 and finalize_prev[0] is not None:
                            finalize_prev[0]()
                            finalize_prev[0] = None
                        if kg == 1:
                            avsm[0] = ps_tile("avsm")
                        if kg >= 1:
                            emit_av(kg - 1)
                            if kg % 2 == 0:
                                emit_colsums(kg // 2 - 1)
                    emit_av(kg_n - 1)
                    emit_colsums(kg_n // 2 - 1)

                    def finalize(avsm=avsm, h=h, qs=qs):
                        av_t = avsm[0]
                        csum = csum_pool.tile([128, 512], F32R, tag="csum")
                        nc.vector.tensor_copy(csum, av_t[:, 1, :])
                        av_sb = av_pool.tile([128, 512], BF16, tag="avsb")
                        nc.scalar.copy(av_sb, av_t[:, 0, :])
                        bc_ps = ps_tile("bcps")
                        nc.tensor.matmul(
                            bc_ps[:, 0, :], sel4, csum, start=True, stop=True
                        )
                        rcp = rcp_pool.tile([128, 512], F32, tag="rcp")
                        nc.vector.reciprocal_approx_fast(
                            out=rcp, in_=bc_ps[:, 0, :]
                        )
                        nc.vector.tensor_mul(outT[:, h, qs], av_sb, rcp)

                    finalize_prev[0] = finalize
            finalize_prev[0]()

            # ---- phase C: output projection ----
            load_wo()
            for scb in range(st_n):
                sb_ = slice(scb * 128, (scb + 1) * 128)
                for dc in range(0, DIM // 512, 2):
                    ops_ = ps_tile("ops")
                    for jj in range(2):
                        for h2 in range(HQ):
                            nc.tensor.matmul(
                                ops_[:, jj, :],
                                outT[:, h2, sb_],
                                wo_sb[:, h2, (dc + jj) * 512:(dc + jj + 1) * 512],
                                start=(h2 == 0),
                                stop=(h2 == HQ - 1),
                            )
                    orow = orow_pool.tile([128, 2, 512], F32, tag="orow")
                    copy_any(orow, ops_)
                    nc.sync.dma_start(
                        out=outp[b, sb_, dc * 512:(dc + 2) * 512],
                        in_=orow.rearrange("p a b -> p (a b)"),
                    )

    nc.compile()
    return nc


_PERM = np.concatenate([np.arange(0, HD, 2), np.arange(1, HD, 2)])


def _prep_inputs(x, freqs, wq, wk, wv, wo, s):
    """Build the 8 per-core input maps."""
    in_maps = []
    xt_dp = []
    cos_dp = []
    sin_dp = []
    for dp in range(DP):
        bs = slice(dp * BPC, (dp + 1) * BPC)
        xt_dp.append(np.ascontiguousarray(x[bs].transpose(0, 2, 1)))
        cos_dp.append(np.ascontiguousarray(np.cos(freqs[bs]).transpose(0, 2, 1)))
        sin_dp.append(np.ascontiguousarray(np.sin(freqs[bs]).transpose(0, 2, 1)))
    for core in range(8):
        g = core % TP
        dp = core // TP
        wq_g = wq[g * HQ * HD:(g + 1) * HQ * HD]  # [512, DIM]
        wq_p = wq_g.reshape(HQ, HD, DIM)[:, _PERM, :].reshape(HQ * HD, DIM)
        wk_g = wk[g * HD:(g + 1) * HD][_PERM]      # [128, DIM]
        wv_g = wv[g * HD:(g + 1) * HD]             # [128, DIM]
        wo_g = wo[:, g * HQ * HD:(g + 1) * HQ * HD]  # [DIM, 512]
        in_maps.append(
            {
                "xt": xt_dp[dp],
                "cost": cos_dp[dp],
                "sint": sin_dp[dp],
                "wqt": np.ascontiguousarray(wq_p.T),
                "wkt": np.ascontiguousarray(wk_g.T),
                "wvt": np.ascontiguousarray(wv_g.T),
                "wot": np.ascontiguousarray(wo_g.T).astype(ml_dtypes.bfloat16),
            }
        )
    return in_maps


_LAST = {}


def _run(x, freqs, wq, wk, wv, wo, s):
    x = np.asarray(x, dtype=np.float32)
    freqs = np.asarray(freqs, dtype=np.float32)
    wq = np.asarray(wq, dtype=np.float32)
    wk = np.asarray(wk, dtype=np.float32)
    wv = np.asarray(wv, dtype=np.float32)
    wo = np.asarray(wo, dtype=np.float32)

    if s not in _NC_CACHE:
        _NC_CACHE[s] = build_nc(s)
    nc = _NC_CACHE[s]
    in_maps = _prep_inputs(x, freqs, wq, wk, wv, wo, s)
    res = run_bass_kernel_spmd(nc, in_maps, core_ids=list(range(8)))
    _LAST["nc"] = nc
    _LAST["in_maps"] = in_maps

    out = np.empty((B, s, DIM), dtype=np.float32)
    for dp in range(DP):
        acc = res.results[dp * TP]["outp"].copy()
        for g in range(1, TP):
            acc += res.results[dp * TP + g]["outp"]
        out[dp * BPC:(dp + 1) * BPC] = acc
    return out


def kernel(x, freqs, wq, wk, wv, wo):
    return _run(x, freqs, wq, wk, wv, wo, S_FULL)
